# revision 39
# baseline (speedup 1.0000x reference)
"""Trainium2 Bass kernel for per-channel sigmoid attention + masked QK^T.

reference:
    logits  = einsum('bic,bjc->bijc', Q, K) + bias          # [B,I,J,C]
    probs   = sigmoid(logits) * mask[..., None]
    output  = probs.sum(-2) / mask.sum(-1, keepdims=True)   # [B,I,C]
    attn    = einsum('bic,bjc->bij', Q, K) * mask           # [B,I,J]

Shapes: B=2, I=1024, J=1024, C=64. Sharded over I across 8 cores
(128 i-rows per core, both batches).

Per-core layout: the 128 SBUF partitions hold the packed (b, c) axis
(c2 = b*64 + c). K is staged as K2T[c2, j] = K[b, j, c]. Per pair of i
values, PSUM logits are accumulated by the PE as
    B: 2^20 * (mask[b,i,j] - 1)   (bf16 selection matmul, lands first so
       in-mask logits see exactly zero)
    A: diag(Q[:, i]) @ K2T        (f32r matmul; the diagonal weight makes
       the per-channel product a matmul)
so that a single ScalarE sigmoid(psum + bias[c]) yields the masked probs
directly (masked-out entries saturate to sigmoid(-2^20) == 0) at fp16
into SBUF, covering two i per instruction. The j-sum is a DVE
tensor_scalar with accum_out (the only reduction with fast fp16 uops).
QK^T is 4 f32r matmuls multiplied by the mask on DVE; the 1/mask.sum()
normalization is broadcast via tiny PE matmuls and applied at the end.
Engine budget per core: ScalarE ~141us (bound), PE ~123us, DVE ~110us,
DMA ~4MB.
"""

import numpy as np
import ml_dtypes

B, I, J, C = 2, 1024, 1024, 64
N_CORES = 8
IL = I // N_CORES  # i-rows per core
P = 128

_PROG_CACHE = {}


def _patch_tile_drain():
    """This walrus build allows one sync-wait per instruction, but
    TileContext emits several where an instruction depends on multiple
    producer procs (and on the tail drain). Split the extras onto
    single-wait NOPs placed immediately before the instruction on the same
    in-order engine queue — semantically identical, since the engine cannot
    pass a queued NOP's wait."""
    import concourse.mybir as mybir
    import concourse.tile as tile
    from concourse.vector_clock import ScopedClock

    if getattr(tile.TileContext, "_drain_patch_applied", False):
        return

    def _split_multiwaits(nc, ordered):
        n_split = 0
        for bb_name, insts in ordered.items():
            new_list = []
            for inst in insts:
                si = getattr(inst, "sync_info", None)
                if si is not None and len(si.on_wait) > 1:
                    waits = list(si.on_wait)
                    for w in waits[:-1]:
                        nop = mybir.InstNoOp(
                            name=nc.get_next_instruction_name(),
                            engine=inst.engine,
                            sync_info=mybir.SyncInfo(on_wait=[w], on_update=[]),
                            bass_nofuse=True,
                        )
                        new_list.append(nop)
                        n_split += 1
                    si.on_wait = [waits[-1]]
                    inst.sync_info = si
                new_list.append(inst)
            insts[:] = new_list
        return n_split

    _orig_lower = tile.TileContext._lower_ordered_insts

    def _lower_ordered_insts(self, ordered):
        _split_multiwaits(self.nc, ordered)
        return _orig_lower(self, ordered)

    tile.TileContext._lower_ordered_insts = _lower_ordered_insts

    def _drain_and_barrier(self, tick_clock, wait_clock):
        drain_inst = self.nc.sync.drain()
        wait_clock.add_sem_waits(
            drain_inst.ins, ScopedClock({None: tick_clock.global_clock})
        )
        si = drain_inst.ins.sync_info
        if si is not None and len(si.on_wait) > 1:
            waits = list(si.on_wait)
            si.on_wait = [waits[0]]
            drain_inst.ins.sync_info = si
            for w in waits[1:]:
                nop = self.nc.sync.nop(nofuse=True)
                nop.ins.sync_info = mybir.SyncInfo(on_wait=[w], on_update=[])
        self.nc.all_engine_barrier()
        assert self.sems is not None
        popped = self.nc._tile_sem_poison_stack.pop()
        assert popped is self._sem_poison
        self.nc.clear_and_free_semaphores(list(self.sems.allocated().values()))
        self.nc.all_engine_barrier()

    tile.TileContext._drain_and_barrier = _drain_and_barrier
    tile.TileContext._drain_patch_applied = True


def build_program(repeat=1):
    """Build the single-core Bass program (same program runs SPMD on all 8
    cores with per-core input shards). repeat>1 re-emits the main compute
    body for wall-clock timing measurements."""
    import concourse.bass as bass
    import concourse.mybir as mybir
    import concourse.tile as tile

    _patch_tile_drain()

    f32 = mybir.dt.float32
    f32r = mybir.dt.float32r
    bf16 = mybir.dt.bfloat16
    f16 = mybir.dt.float16
    SIG = mybir.ActivationFunctionType.Sigmoid
    MULT = mybir.AluOpType.mult
    ADD = mybir.AluOpType.add
    AXX = mybir.AxisListType.X

    nc = bass.Bass("TRN2", target_bir_lowering=False, debug=False)

    # ---- DRAM parameters ----
    d_k2t = nc.dram_tensor("k2t", [P, J], f32r, kind="ExternalInput").ap()
    d_qs = nc.dram_tensor("qs", [P, IL], f32, kind="ExternalInput").ap()
    d_qs0 = nc.dram_tensor("qs0", [P, IL], f32r, kind="ExternalInput").ap()
    d_qs1 = nc.dram_tensor("qs1", [P, IL], f32r, kind="ExternalInput").ap()
    d_biasT = nc.dram_tensor("biasT", [P, 1], f32, kind="ExternalInput").ap()
    d_ones64 = nc.dram_tensor("ones64", [1, C], f32, kind="ExternalInput").ap()
    d_selall = nc.dram_tensor("selall", [P, 64 * P], bf16, kind="ExternalInput").ap()
    d_ident = nc.dram_tensor("ident", [P, P], f32, kind="ExternalInput").ap()
    d_mask0 = nc.dram_tensor("mask0", [P, J], bf16, kind="ExternalInput").ap()
    d_mask1 = nc.dram_tensor("mask1", [P, J], bf16, kind="ExternalInput").ap()
    # pair-interleaved (mask - 1) in {-1, 0}: row 2*il+b = mask[b, il, :] - 1
    d_mm1A = nc.dram_tensor("mm1A", [P, J], bf16, kind="ExternalInput").ap()
    d_mm1B = nc.dram_tensor("mm1B", [P, J], bf16, kind="ExternalInput").ap()

    d_out = nc.dram_tensor("out_t", [P, IL], f32, kind="ExternalOutput").ap()
    d_attn0 = nc.dram_tensor("attn0", [P, J], f32, kind="ExternalOutput").ap()
    d_attn1 = nc.dram_tensor("attn1", [P, J], f32, kind="ExternalOutput").ap()

    with tile.TileContext(nc) as tc:
        with tc.tile_pool(name="consts", bufs=1) as cpool:
            # Persistent SBUF inputs, loop-critical DMAs first. k2t and
            # selall gate the first logits matmuls, so they are split into
            # chunks: group g only waits for the chunk covering its slice
            # (Tile tracks deps per DMA instruction).
            k2t = cpool.tile([P, J], f32r)
            for jh in range(2):
                nc.sync.dma_start(
                    k2t[:, jh * 512:(jh + 1) * 512],
                    d_k2t[:, jh * 512:(jh + 1) * 512],
                )
            qs = cpool.tile([P, IL], f32)
            nc.sync.dma_start(qs[:], d_qs[:])
            biasT = cpool.tile([P, 1], f32)
            nc.sync.dma_start(biasT[:], d_biasT[:])
            selall = cpool.tile([P, 64 * P], bf16)
            for ch in range(8):
                lo = ch * 8 * P
                nc.sync.dma_start(
                    selall[:, lo:lo + 8 * P], d_selall[:, lo:lo + 8 * P]
                )
            mm1A = cpool.tile([P, J], bf16)
            nc.sync.dma_start(mm1A[:], d_mm1A[:])
            mm1B = cpool.tile([P, J], bf16)
            nc.sync.dma_start(mm1B[:], d_mm1B[:])
            qs0 = cpool.tile([P, IL], f32r)
            nc.sync.dma_start(qs0[:], d_qs0[:])
            qs1 = cpool.tile([P, IL], f32r)
            nc.sync.dma_start(qs1[:], d_qs1[:])
            ones64 = cpool.tile([1, C], f32)
            nc.sync.dma_start(ones64[:], d_ones64[:])
            ident = cpool.tile([P, P], f32)
            nc.sync.dma_start(ident[:], d_ident[:])
            mask0 = cpool.tile([P, J], bf16)
            nc.sync.dma_start(mask0[:], d_mask0[:])
            mask1 = cpool.tile([P, J], bf16)
            nc.sync.dma_start(mask1[:], d_mask1[:])

            OUT = cpool.tile([P, IL], f32)     # unnormalized output columns
            r0 = cpool.tile([1, IL], f32)      # 1/msum row, batch 0
            r1 = cpool.tile([1, IL], f32)      # 1/msum row, batch 1

            def emit_body():
                # One shared PSUM pool: two rotating 4-bank slots. Prologue
                # psum tiles (qk, psumT, rpF) are short-lived (consumed into
                # SBUF immediately), so they flow through the same slots the
                # main loop uses — no pool-scope barrier between phases.
                with (
                    tc.tile_pool(name="ps", bufs=2, space="PSUM") as ppool,
                    tc.tile_pool(name="sg", bufs=4) as spool,
                    tc.tile_pool(name="dg", bufs=6) as dpool,
                    tc.tile_pool(name="scr", bufs=3) as scpool,
                ):
                    # ---- Prologue: QK^T, attn = QK^T*mask, msum, recipF ----
                    for b, (qsb, maskb, d_attnb) in enumerate(
                        [(qs0, mask0, d_attn0), (qs1, mask1, d_attn1)]
                    ):
                        qk = ppool.tile([P, J], f32, tag="slot")
                        for jh in range(2):
                            nc.tensor.matmul(
                                qk[:, jh * 512:(jh + 1) * 512],
                                qsb[:],
                                k2t[:, jh * 512:(jh + 1) * 512],
                                start=True,
                                stop=True,
                            )
                        attn_sb = cpool.tile([P, J], f32, tag=f"attn_sb{b}")
                        nc.vector.tensor_tensor(attn_sb[:], qk[:], maskb[:], MULT)
                        nc.sync.dma_start(d_attnb[:], attn_sb[:])

                    for b, (maskb, rb) in enumerate([(mask0, r0), (mask1, r1)]):
                        msum = cpool.tile([P, 1], f32, tag=f"msum{b}")
                        nc.vector.tensor_reduce(msum[:], maskb[:], axis=AXX, op=ADD)
                        mrec = cpool.tile([P, 1], f32, tag=f"mrec{b}")
                        nc.vector.reciprocal(mrec[:], msum[:])
                        psumT = ppool.tile([1, P], f32, tag="slot")
                        nc.tensor.matmul(
                            psumT[:],
                            mrec[:],
                            ident[:],
                            start=True,
                            stop=True,
                        )
                        nc.vector.tensor_copy(rb[:], psumT[:])

                    # recipF[c2, i] = 1/msum[b(c2), i]: each batch's 1/msum
                    # row broadcast onto its 64 channel partitions (PE out
                    # offsets 0 and 64), parked in SBUF.
                    recipF = cpool.tile([P, IL], f32)
                    rpF = ppool.tile([P, IL], f32, tag="slot")
                    for b, rb in enumerate((r0, r1)):
                        nc.tensor.matmul(
                            rpF[b * C:(b + 1) * C, :],
                            ones64[:],
                            rb[:],
                            start=True,
                            stop=True,
                        )
                    nc.vector.tensor_copy(recipF[:], rpF[:])

                    # ---- Main loop: 64 groups of two i values ----
                    # logits_psum[c2, j] accumulates, in order:
                    #   B: 2^20 * (mask[b, i, j] - 1)   (bf16 matmul, lands
                    #      first so in-mask logits see exactly 0)
                    #   A: Q[b, i, c] * K2T[c2, j]      (f32r diag matmul)
                    # One ScalarE sigmoid covers both i of a group (scale/
                    # bias don't depend on i), writing fp16 to SBUF;
                    # masked-out entries saturate to sigmoid(-2^20) == 0.
                    # DVE reduces each i at fp16 fast rate into OUT.
                    for g in range(IL // 2):
                        lg = ppool.tile([P, 2 * J], f32, tag="slot")
                        diags = []
                        for h in range(2):
                            i = 2 * g + h
                            pairT = mm1A if i < 64 else mm1B
                            il = i % 64
                            # selall slice cols: e_{2il}*2^20 x64 then
                            # e_{2il+1}*2^20 x64 -> row b*64+c picks
                            # 2^20*(mask[b,i,j]-1) from the pair tile
                            sel = selall[:, il * P:(il + 1) * P]
                            diag = dpool.tile([P, P], f32r)
                            nc.vector.tensor_scalar(
                                diag[:], ident[:], qs[:, i:i + 1], None, MULT
                            )
                            diags.append(diag)
                            for jh in range(2):
                                lo = h * J + jh * 512
                                nc.tensor.matmul(
                                    lg[:, lo:lo + 512],
                                    sel,
                                    pairT[:, jh * 512:(jh + 1) * 512],
                                    start=True,
                                    stop=False,
                                )
                                nc.tensor.matmul(
                                    lg[:, lo:lo + 512],
                                    diag[:],
                                    k2t[:, jh * 512:(jh + 1) * 512],
                                    start=False,
                                    stop=True,
                                )
                        s = spool.tile([P, 2 * J], f16)
                        nc.scalar.activation(s[:], lg[:], SIG, bias=biasT[:, 0:1])
                        for h in range(2):
                            i = 2 * g + h
                            # tensor_scalar (4x fp16 mode) with accum_out is
                            # the fast path for a free-dim sum; tensor_reduce
                            # has no 2x/4x uops.
                            scr = scpool.tile([P, J], f16)
                            nc.vector.tensor_scalar(
                                scr[:],
                                s[:, h * J:(h + 1) * J],
                                1.0,
                                None,
                                op0=MULT,
                                op1=ADD,
                                accum_out=OUT[:, i:i + 1],
                            )

                # ---- Epilogue: normalize and store ----
                outF = cpool.tile([P, IL], f32)
                nc.vector.tensor_tensor(outF[:], OUT[:], recipF[:], MULT)
                nc.sync.dma_start(d_out[:], outF[:])

            if repeat == 1:
                emit_body()
            else:
                # Body exceeds one IRAM block on PE/DVE; hint the back-edge
                # so the branch target prefetches instead of stalling ~4us.
                with tc.For_i(
                    0, repeat, 1,
                    hint_engines=(
                        mybir.EngineType.PE,
                        mybir.EngineType.DVE,
                        mybir.EngineType.SP,
                    ),
                ):
                    emit_body()

    return nc


def _shard_inputs(Q, K, bias, mask):
    """Host-side layout prep: pure transpose/reshape/cast, no arithmetic."""
    f32 = np.float32
    bf16 = ml_dtypes.bfloat16
    Q = np.ascontiguousarray(Q, dtype=f32)
    K = np.ascontiguousarray(K, dtype=f32)
    bias = np.ascontiguousarray(bias, dtype=f32)
    mask = np.ascontiguousarray(mask, dtype=f32)

    # Shared across cores.
    k2t = K.transpose(0, 2, 1).reshape(P, J).copy()          # [b*64+c, j]
    biasT = np.concatenate([bias, bias]).reshape(P, 1).copy()
    ones64 = np.ones((1, C), dtype=f32)
    ident = np.eye(P, dtype=f32)
    # selall[:, il*128:(il+1)*128] = selection matrix for local row il:
    # column b*64+c is 2^20 * e_{2il+b} (exact in bf16)
    selall = np.zeros((P, 64, 2, C), dtype=np.float32)
    for il in range(64):
        selall[2 * il, il, 0, :] = 2.0**20
        selall[2 * il + 1, il, 1, :] = 2.0**20
    selall = selall.reshape(P, 64 * P).astype(bf16)

    in_maps = []
    for m in range(N_CORES):
        sl = slice(m * IL, (m + 1) * IL)
        Qm = Q[:, sl, :]                                      # [2, 128, 64]
        qs = Qm.transpose(0, 2, 1).reshape(P, IL).copy()      # [b*64+c, il]
        qs0 = qs.copy()
        qs0[64:] = 0.0
        qs1 = qs.copy()
        qs1[:64] = 0.0
        Mm = mask[:, sl, :]                                   # [2, 128, 1024]
        mask_b = Mm.astype(bf16)
        # pair-interleaved (mask-1) in {-1, 0}: row 2*il+b = mask[b, .., :]-1
        Mm1 = Mm - np.float32(1.0)
        mm1A = Mm1[:, :64, :].transpose(1, 0, 2).reshape(P, J).astype(bf16)
        mm1B = Mm1[:, 64:, :].transpose(1, 0, 2).reshape(P, J).astype(bf16)
        in_maps.append({
            "k2t": k2t,
            "qs": qs,
            "qs0": qs0,
            "qs1": qs1,
            "biasT": biasT,
            "ones64": ones64,
            "selall": selall,
            "ident": ident,
            "mask0": np.ascontiguousarray(mask_b[0]),
            "mask1": np.ascontiguousarray(mask_b[1]),
            "mm1A": np.ascontiguousarray(mm1A),
            "mm1B": np.ascontiguousarray(mm1B),
        })
    return in_maps


def _gather_outputs(results):
    output = np.empty((B, I, C), dtype=np.float32)
    attn = np.empty((B, I, J), dtype=np.float32)
    for m in range(N_CORES):
        sl = slice(m * IL, (m + 1) * IL)
        out_t = results[m]["out_t"]                       # [128 c2, 128 il]
        output[:, sl, :] = out_t.reshape(B, C, IL).transpose(0, 2, 1)
        attn[0, sl, :] = results[m]["attn0"]
        attn[1, sl, :] = results[m]["attn1"]
    return output, attn


def kernel(Q, K, bias, mask):
    import time

    from concourse.bass_utils import run_bass_kernel_spmd

    if "nc" not in _PROG_CACHE:
        _PROG_CACHE["nc"] = build_program()
    nc = _PROG_CACHE["nc"]
    in_maps = _shard_inputs(Q, K, bias, mask)
    try:
        res = run_bass_kernel_spmd(nc, in_maps, list(range(N_CORES)))
    except Exception:
        # The shared axon device occasionally reports a transient
        # NRT_EXEC_UNIT_UNRECOVERABLE; one retry clears it.
        time.sleep(2.0)
        res = run_bass_kernel_spmd(nc, in_maps, list(range(N_CORES)))
    return _gather_outputs(res.results)


# revision 40
# speedup vs baseline: 1.2676x; 1.2676x over previous
"""Trainium2 Bass kernel for per-channel sigmoid attention + masked QK^T.

reference:
    logits  = einsum('bic,bjc->bijc', Q, K) + bias          # [B,I,J,C]
    probs   = sigmoid(logits) * mask[..., None]
    output  = probs.sum(-2) / mask.sum(-1, keepdims=True)   # [B,I,C]
    attn    = einsum('bic,bjc->bij', Q, K) * mask           # [B,I,J]

Shapes: B=2, I=1024, J=1024, C=64. Sharded over I across 8 cores
(128 i-rows per core, both batches).

Per-core layout: the 128 SBUF partitions hold the packed (b, c) axis
(c2 = b*64 + c). K is staged as K2T[c2, j] = K[b, j, c]. Per pair of i
values, PSUM logits are accumulated by the PE as
    B: 2^20 * (mask[b,i,j] - 1)   (bf16 selection matmul, lands first so
       in-mask logits see exactly zero)
    A: diag(Q[:, i]) @ K2T        (f32r matmul; the diagonal weight makes
       the per-channel product a matmul)
so that a single ScalarE sigmoid(psum + bias[c]) yields the masked probs
directly (masked-out entries saturate to sigmoid(-2^20) == 0) at fp16
into SBUF, covering two i per instruction. The j-sum is a DVE
tensor_scalar with accum_out (the only reduction with fast fp16 uops).
QK^T is 4 f32r matmuls multiplied by the mask on DVE; the 1/mask.sum()
normalization is broadcast via tiny PE matmuls and applied at the end.
Engine budget per core: ScalarE ~141us (bound), PE ~123us, DVE ~110us,
DMA ~4MB.
"""

import numpy as np
import ml_dtypes

B, I, J, C = 2, 1024, 1024, 64
N_CORES = 8
IL = I // N_CORES  # i-rows per core
P = 128

_PROG_CACHE = {}


def _patch_tile_drain():
    """This walrus build allows one sync-wait per instruction, but
    TileContext emits several where an instruction depends on multiple
    producer procs (and on the tail drain). Split the extras onto
    single-wait NOPs placed immediately before the instruction on the same
    in-order engine queue — semantically identical, since the engine cannot
    pass a queued NOP's wait."""
    import concourse.mybir as mybir
    import concourse.tile as tile
    from concourse.vector_clock import ScopedClock

    if getattr(tile.TileContext, "_drain_patch_applied", False):
        return

    def _split_multiwaits(nc, ordered):
        n_split = 0
        for bb_name, insts in ordered.items():
            new_list = []
            for inst in insts:
                si = getattr(inst, "sync_info", None)
                if si is not None and len(si.on_wait) > 1:
                    waits = list(si.on_wait)
                    for w in waits[:-1]:
                        nop = mybir.InstNoOp(
                            name=nc.get_next_instruction_name(),
                            engine=inst.engine,
                            sync_info=mybir.SyncInfo(on_wait=[w], on_update=[]),
                            bass_nofuse=True,
                        )
                        new_list.append(nop)
                        n_split += 1
                    si.on_wait = [waits[-1]]
                    inst.sync_info = si
                new_list.append(inst)
            insts[:] = new_list
        return n_split

    _orig_lower = tile.TileContext._lower_ordered_insts

    def _lower_ordered_insts(self, ordered):
        _split_multiwaits(self.nc, ordered)
        return _orig_lower(self, ordered)

    tile.TileContext._lower_ordered_insts = _lower_ordered_insts

    def _drain_and_barrier(self, tick_clock, wait_clock):
        drain_inst = self.nc.sync.drain()
        wait_clock.add_sem_waits(
            drain_inst.ins, ScopedClock({None: tick_clock.global_clock})
        )
        si = drain_inst.ins.sync_info
        if si is not None and len(si.on_wait) > 1:
            waits = list(si.on_wait)
            si.on_wait = [waits[0]]
            drain_inst.ins.sync_info = si
            for w in waits[1:]:
                nop = self.nc.sync.nop(nofuse=True)
                nop.ins.sync_info = mybir.SyncInfo(on_wait=[w], on_update=[])
        self.nc.all_engine_barrier()
        assert self.sems is not None
        popped = self.nc._tile_sem_poison_stack.pop()
        assert popped is self._sem_poison
        self.nc.clear_and_free_semaphores(list(self.sems.allocated().values()))
        self.nc.all_engine_barrier()

    tile.TileContext._drain_and_barrier = _drain_and_barrier
    tile.TileContext._drain_patch_applied = True


def build_program(repeat=1):
    """Build the single-core Bass program (same program runs SPMD on all 8
    cores with per-core input shards). repeat>1 re-emits the main compute
    body for wall-clock timing measurements."""
    import concourse.bass as bass
    import concourse.mybir as mybir
    import concourse.tile as tile

    _patch_tile_drain()

    f32 = mybir.dt.float32
    f32r = mybir.dt.float32r
    bf16 = mybir.dt.bfloat16
    f16 = mybir.dt.float16
    SIG = mybir.ActivationFunctionType.Sigmoid
    MULT = mybir.AluOpType.mult
    ADD = mybir.AluOpType.add
    AXX = mybir.AxisListType.X

    nc = bass.Bass("TRN2", target_bir_lowering=False, debug=False)

    # ---- DRAM parameters ----
    d_k2t = nc.dram_tensor("k2t", [P, J], f32r, kind="ExternalInput").ap()
    d_qs = nc.dram_tensor("qs", [P, IL], f32, kind="ExternalInput").ap()
    d_qs0 = nc.dram_tensor("qs0", [P, IL], f32r, kind="ExternalInput").ap()
    d_qs1 = nc.dram_tensor("qs1", [P, IL], f32r, kind="ExternalInput").ap()
    d_biasT = nc.dram_tensor("biasT", [P, 1], f32, kind="ExternalInput").ap()
    d_ones64 = nc.dram_tensor("ones64", [1, C], f32, kind="ExternalInput").ap()
    d_selall = nc.dram_tensor("selall", [P, 64 * P], bf16, kind="ExternalInput").ap()
    d_ident = nc.dram_tensor("ident", [P, P], f32, kind="ExternalInput").ap()
    d_mask0 = nc.dram_tensor("mask0", [P, J], bf16, kind="ExternalInput").ap()
    d_mask1 = nc.dram_tensor("mask1", [P, J], bf16, kind="ExternalInput").ap()
    # pair-interleaved (mask - 1) in {-1, 0}: row 2*il+b = mask[b, il, :] - 1
    d_mm1A = nc.dram_tensor("mm1A", [P, J], bf16, kind="ExternalInput").ap()
    d_mm1B = nc.dram_tensor("mm1B", [P, J], bf16, kind="ExternalInput").ap()

    d_out = nc.dram_tensor("out_t", [P, IL], f32, kind="ExternalOutput").ap()
    d_attn0 = nc.dram_tensor("attn0", [P, J], f32, kind="ExternalOutput").ap()
    d_attn1 = nc.dram_tensor("attn1", [P, J], f32, kind="ExternalOutput").ap()

    with tile.TileContext(nc) as tc:
        with tc.tile_pool(name="consts", bufs=1) as cpool:
            # Persistent SBUF inputs, loop-critical DMAs first. k2t and
            # selall gate the first logits matmuls, so they are split into
            # chunks: group g only waits for the chunk covering its slice
            # (Tile tracks deps per DMA instruction).
            k2t = cpool.tile([P, J], f32r)
            for jh in range(2):
                nc.sync.dma_start(
                    k2t[:, jh * 512:(jh + 1) * 512],
                    d_k2t[:, jh * 512:(jh + 1) * 512],
                )
            qs = cpool.tile([P, IL], f32)
            nc.sync.dma_start(qs[:], d_qs[:])
            biasT = cpool.tile([P, 1], f32)
            nc.sync.dma_start(biasT[:], d_biasT[:])
            selall = cpool.tile([P, 64 * P], bf16)
            for ch in range(8):
                lo = ch * 8 * P
                nc.sync.dma_start(
                    selall[:, lo:lo + 8 * P], d_selall[:, lo:lo + 8 * P]
                )
            mm1A = cpool.tile([P, J], bf16)
            nc.sync.dma_start(mm1A[:], d_mm1A[:])
            mm1B = cpool.tile([P, J], bf16)
            nc.sync.dma_start(mm1B[:], d_mm1B[:])
            qs0 = cpool.tile([P, IL], f32r)
            nc.sync.dma_start(qs0[:], d_qs0[:])
            qs1 = cpool.tile([P, IL], f32r)
            nc.sync.dma_start(qs1[:], d_qs1[:])
            ones64 = cpool.tile([1, C], f32)
            nc.sync.dma_start(ones64[:], d_ones64[:])
            ident = cpool.tile([P, P], f32)
            nc.sync.dma_start(ident[:], d_ident[:])
            mask0 = cpool.tile([P, J], bf16)
            nc.sync.dma_start(mask0[:], d_mask0[:])
            mask1 = cpool.tile([P, J], bf16)
            nc.sync.dma_start(mask1[:], d_mask1[:])

            OUT = cpool.tile([P, IL], f32)     # unnormalized output columns
            r0 = cpool.tile([1, IL], f32)      # 1/msum row, batch 0
            r1 = cpool.tile([1, IL], f32)      # 1/msum row, batch 1

            def emit_body():
                # One shared PSUM pool: two rotating 4-bank slots. Prologue
                # psum tiles (qk, psumT, rpF) are short-lived (consumed into
                # SBUF immediately), so they flow through the same slots the
                # main loop uses — no pool-scope barrier between phases.
                with (
                    tc.tile_pool(name="ps", bufs=2, space="PSUM") as ppool,
                    tc.tile_pool(name="sg", bufs=4) as spool,
                    tc.tile_pool(name="dg", bufs=6) as dpool,
                    tc.tile_pool(name="scr", bufs=3) as scpool,
                ):
                    # ---- Prologue: QK^T, attn = QK^T*mask, msum, recipF ----
                    for b, (qsb, maskb, d_attnb) in enumerate(
                        [(qs0, mask0, d_attn0), (qs1, mask1, d_attn1)]
                    ):
                        qk = ppool.tile([P, J], f32, tag="slot")
                        for jh in range(2):
                            nc.tensor.matmul(
                                qk[:, jh * 512:(jh + 1) * 512],
                                qsb[:],
                                k2t[:, jh * 512:(jh + 1) * 512],
                                start=True,
                                stop=True,
                            )
                        attn_sb = cpool.tile([P, J], f32, tag=f"attn_sb{b}")
                        nc.vector.tensor_tensor(attn_sb[:], qk[:], maskb[:], MULT)
                        nc.sync.dma_start(d_attnb[:], attn_sb[:])

                    for b, (maskb, rb) in enumerate([(mask0, r0), (mask1, r1)]):
                        msum = cpool.tile([P, 1], f32, tag=f"msum{b}")
                        nc.vector.tensor_reduce(msum[:], maskb[:], axis=AXX, op=ADD)
                        mrec = cpool.tile([P, 1], f32, tag=f"mrec{b}")
                        nc.vector.reciprocal(mrec[:], msum[:])
                        psumT = ppool.tile([1, P], f32, tag="slot")
                        nc.tensor.matmul(
                            psumT[:],
                            mrec[:],
                            ident[:],
                            start=True,
                            stop=True,
                        )
                        nc.vector.tensor_copy(rb[:], psumT[:])

                    # recipF[c2, i] = 1/msum[b(c2), i]: each batch's 1/msum
                    # row broadcast onto its 64 channel partitions (PE out
                    # offsets 0 and 64), parked in SBUF.
                    recipF = cpool.tile([P, IL], f32)
                    rpF = ppool.tile([P, IL], f32, tag="slot")
                    for b, rb in enumerate((r0, r1)):
                        nc.tensor.matmul(
                            rpF[b * C:(b + 1) * C, :],
                            ones64[:],
                            rb[:],
                            start=True,
                            stop=True,
                        )
                    nc.vector.tensor_copy(recipF[:], rpF[:])

                    # ---- Main loop: 64 groups of two i values ----
                    # logits_psum[c2, j] accumulates, in order:
                    #   B: 2^20 * (mask[b, i, j] - 1)   (bf16 matmul, lands
                    #      first so in-mask logits see exactly 0)
                    #   A: Q[b, i, c] * K2T[c2, j]      (f32r diag matmul)
                    # One ScalarE sigmoid covers both i of a group (scale/
                    # bias don't depend on i), writing fp16 to SBUF;
                    # masked-out entries saturate to sigmoid(-2^20) == 0.
                    # DVE reduces each i at fp16 fast rate into OUT.
                    for g in range(IL // 2):
                        lg = ppool.tile([P, 2 * J], f32, tag="slot")
                        diags = []
                        for h in range(2):
                            i = 2 * g + h
                            diag = dpool.tile([P, P], f32r)
                            nc.vector.tensor_scalar(
                                diag[:], ident[:], qs[:, i:i + 1], None, MULT
                            )
                            diags.append(diag)
                        # All four bf16 B matmuls, then all four f32r A
                        # matmuls: 2 PE weight-mode switches per group
                        # instead of 8. B closes its accumulation group
                        # (start+stop) and A re-accumulates onto the bank
                        # (start=False, skip_group_check) — per-bank order
                        # B-before-A is preserved, so in-mask logits still
                        # see exactly zero from the B pass.
                        for h in range(2):
                            i = 2 * g + h
                            pairT = mm1A if i < 64 else mm1B
                            il = i % 64
                            # selall slice cols: e_{2il}*2^20 x64 then
                            # e_{2il+1}*2^20 x64 -> row b*64+c picks
                            # 2^20*(mask[b,i,j]-1) from the pair tile
                            sel = selall[:, il * P:(il + 1) * P]
                            for jh in range(2):
                                lo = h * J + jh * 512
                                nc.tensor.matmul(
                                    lg[:, lo:lo + 512],
                                    sel,
                                    pairT[:, jh * 512:(jh + 1) * 512],
                                    start=True,
                                    stop=True,
                                )
                        for h in range(2):
                            for jh in range(2):
                                lo = h * J + jh * 512
                                nc.tensor.matmul(
                                    lg[:, lo:lo + 512],
                                    diags[h][:],
                                    k2t[:, jh * 512:(jh + 1) * 512],
                                    start=False,
                                    stop=True,
                                    skip_group_check=True,
                                )
                        s = spool.tile([P, 2 * J], f16)
                        nc.scalar.activation(s[:], lg[:], SIG, bias=biasT[:, 0:1])
                        for h in range(2):
                            i = 2 * g + h
                            # tensor_scalar (4x fp16 mode) with accum_out is
                            # the fast path for a free-dim sum; tensor_reduce
                            # has no 2x/4x uops.
                            scr = scpool.tile([P, J], f16)
                            nc.vector.tensor_scalar(
                                scr[:],
                                s[:, h * J:(h + 1) * J],
                                1.0,
                                None,
                                op0=MULT,
                                op1=ADD,
                                accum_out=OUT[:, i:i + 1],
                            )

                # ---- Epilogue: normalize and store ----
                outF = cpool.tile([P, IL], f32)
                nc.vector.tensor_tensor(outF[:], OUT[:], recipF[:], MULT)
                nc.sync.dma_start(d_out[:], outF[:])

            if repeat == 1:
                emit_body()
            else:
                # Body exceeds one IRAM block on PE/DVE; hint the back-edge
                # so the branch target prefetches instead of stalling ~4us.
                with tc.For_i(
                    0, repeat, 1,
                    hint_engines=(
                        mybir.EngineType.PE,
                        mybir.EngineType.DVE,
                        mybir.EngineType.SP,
                    ),
                ):
                    emit_body()

    return nc


def _shard_inputs(Q, K, bias, mask):
    """Host-side layout prep: pure transpose/reshape/cast, no arithmetic."""
    f32 = np.float32
    bf16 = ml_dtypes.bfloat16
    Q = np.ascontiguousarray(Q, dtype=f32)
    K = np.ascontiguousarray(K, dtype=f32)
    bias = np.ascontiguousarray(bias, dtype=f32)
    mask = np.ascontiguousarray(mask, dtype=f32)

    # Shared across cores.
    k2t = K.transpose(0, 2, 1).reshape(P, J).copy()          # [b*64+c, j]
    biasT = np.concatenate([bias, bias]).reshape(P, 1).copy()
    ones64 = np.ones((1, C), dtype=f32)
    ident = np.eye(P, dtype=f32)
    # selall[:, il*128:(il+1)*128] = selection matrix for local row il:
    # column b*64+c is 2^20 * e_{2il+b} (exact in bf16)
    selall = np.zeros((P, 64, 2, C), dtype=np.float32)
    for il in range(64):
        selall[2 * il, il, 0, :] = 2.0**20
        selall[2 * il + 1, il, 1, :] = 2.0**20
    selall = selall.reshape(P, 64 * P).astype(bf16)

    in_maps = []
    for m in range(N_CORES):
        sl = slice(m * IL, (m + 1) * IL)
        Qm = Q[:, sl, :]                                      # [2, 128, 64]
        qs = Qm.transpose(0, 2, 1).reshape(P, IL).copy()      # [b*64+c, il]
        qs0 = qs.copy()
        qs0[64:] = 0.0
        qs1 = qs.copy()
        qs1[:64] = 0.0
        Mm = mask[:, sl, :]                                   # [2, 128, 1024]
        mask_b = Mm.astype(bf16)
        # pair-interleaved (mask-1) in {-1, 0}: row 2*il+b = mask[b, .., :]-1
        Mm1 = Mm - np.float32(1.0)
        mm1A = Mm1[:, :64, :].transpose(1, 0, 2).reshape(P, J).astype(bf16)
        mm1B = Mm1[:, 64:, :].transpose(1, 0, 2).reshape(P, J).astype(bf16)
        in_maps.append({
            "k2t": k2t,
            "qs": qs,
            "qs0": qs0,
            "qs1": qs1,
            "biasT": biasT,
            "ones64": ones64,
            "selall": selall,
            "ident": ident,
            "mask0": np.ascontiguousarray(mask_b[0]),
            "mask1": np.ascontiguousarray(mask_b[1]),
            "mm1A": np.ascontiguousarray(mm1A),
            "mm1B": np.ascontiguousarray(mm1B),
        })
    return in_maps


def _gather_outputs(results):
    output = np.empty((B, I, C), dtype=np.float32)
    attn = np.empty((B, I, J), dtype=np.float32)
    for m in range(N_CORES):
        sl = slice(m * IL, (m + 1) * IL)
        out_t = results[m]["out_t"]                       # [128 c2, 128 il]
        output[:, sl, :] = out_t.reshape(B, C, IL).transpose(0, 2, 1)
        attn[0, sl, :] = results[m]["attn0"]
        attn[1, sl, :] = results[m]["attn1"]
    return output, attn


def kernel(Q, K, bias, mask):
    import time

    from concourse.bass_utils import run_bass_kernel_spmd

    if "nc" not in _PROG_CACHE:
        _PROG_CACHE["nc"] = build_program()
    nc = _PROG_CACHE["nc"]
    in_maps = _shard_inputs(Q, K, bias, mask)
    try:
        res = run_bass_kernel_spmd(nc, in_maps, list(range(N_CORES)))
    except Exception:
        # The shared axon device occasionally reports a transient
        # NRT_EXEC_UNIT_UNRECOVERABLE; one retry clears it.
        time.sleep(2.0)
        res = run_bass_kernel_spmd(nc, in_maps, list(range(N_CORES)))
    return _gather_outputs(res.results)
